# revision 31
# baseline (speedup 1.0000x reference)
"""Single-head attention (B=4, S=2048, E=1024, H=64) on 8 TRN2 NeuronCores.

Sharding: each batch b is handled by a core pair; core 2b takes keys/values
[0:1024), core 2b+1 takes [1024:2048). Each core computes, for ALL 2048
queries of its batch, the unnormalized attention numerator and denominator
over its key half; the host sums the two halves and divides.

v3 structure (packed PE array -- the v2 trace showed the PE 95% busy with
half-empty matmuls while exp dribbled behind it):
  - k/q projections use duplicated-output weights [W|W] (lhsT [128,128]) so
    kT/qT land at partitions 0:64 AND 64:128 in the same 512-cycle matmul.
  - scores run as ROW-TILED pairs: chunk 2si contracts on PE rows 0:63,
    chunk 2si+1 on rows 64:127 concurrently (tile_position auto-derived
    from base partitions) -- 2x score throughput.
  - v projection runs as a COL-TILED pair (block 0 -> psum[0:64], block 1
    -> psum[64:128] concurrently).
  - exp ACTIVATEs write contiguous [128,1024] runs (expT laid out
    [P, si, 2, 512] per query block).
DMA: 16 fine-grained transfers, priority aux | k0 | q0 | k1 | q1 | q2 | v
| q3 split across the sync/scalar HWDGE queues by e-chunk halves
(~400GB/s aggregate measured). exp0 starts as soon as k0+q0+proj are done
(~18us); the ACT queue frees after its 8 dma_start issues (~16us).
"""

import numpy as np

_B, _S, _E, _H = 4, 2048, 1024, 64
_P = 128
_EC = _E // _P          # 8 E-chunks
_SK = _S // 2           # 1024 keys per core
_SKC = _SK // _P        # 8 sk chunks
_NQ = _S // 512         # 4 query 512-blocks
_NWU = 56               # PE keep-warm dummy matmuls (bridge to k0 arrival)

_built = None


def _build():
    import concourse.bacc as bacc
    import concourse.mybir as mybir
    import concourse.tile as tile

    f32 = mybir.dt.float32
    f16 = mybir.dt.float16
    bf16 = mybir.dt.bfloat16
    Exp = mybir.ActivationFunctionType.Exp

    nc = bacc.Bacc("TRN2", target_bir_lowering=False, debug=False,
                   enable_asserts=False, num_devices=8)

    aux_d = nc.dram_tensor("aux", [_P, 25, 64], f16, kind="ExternalInput")
    bs_d = nc.dram_tensor("bs", [_P, 3], f32, kind="ExternalInput")
    xk_d = nc.dram_tensor("xk", [_P, 2, _EC, 512], f16, kind="ExternalInput")
    xq_d = nc.dram_tensor("xq", [_NQ, _P, _EC, 512], f16, kind="ExternalInput")
    xv_d = nc.dram_tensor("xv", [_P, _EC, _SK], f16, kind="ExternalInput")
    out_d = nc.dram_tensor("out", [_H + 1, _S], f32, kind="ExternalOutput")

    with tile.TileContext(nc) as tc:
        with (
            tc.tile_pool(name="persist", bufs=1) as persist,
        ):
            aux_sb = persist.tile([_P, 25, 64], f16)
            bs_sb = persist.tile([_P, 3], f32)
            w2_sb = persist.tile([_P, 3, _EC, 128], f16)

            xk_sb = persist.tile([_P, 2, _EC, 512], f16)
            xq_sb = [persist.tile([_P, _EC, 512], f16, name=f"xq{j}")
                     for j in range(_NQ)]
            xv_sb = persist.tile([_P, _EC, _SK], f16)

            # kq2: rows 0:64 and 64:128 both hold kT|qT (dup via [W|W]).
            # cols: k0T 0:512 | k1T 512:1024 | qT 1024:3072
            kq2 = persist.tile([_P, _SK + _S], f16)
            vT2 = persist.tile([_P, 512], f16)   # rows 0:64 = vT[:,0:512]
            v_sb = persist.tile([_P, _SKC, _H + 1], bf16)
            expT = [persist.tile([_P, 4, 2, 512], bf16, name=f"ex{j}")
                    for j in range(_NQ)]
            oT_sb = persist.tile([_H + 1, _S], f32)

            wu_sb = persist.tile([_P, 512], f16)
            wu_act = persist.tile([_P, 32], bf16)
            nc.vector.memset(wu_sb[:], 0.0)
            nc.vector.memset(v_sb[:, :, _H:_H + 1], 1.0)

            # Input DMAs: fine-grained, priority order aux|k0|q0|k1|q1|q2|v|q3,
            # e-chunks 0:4 on sync, 4:8 on scalar. The scalar (ACT) queue
            # gets only 7 issues whose DMAHW-lane predecessors complete
            # early, so the ACT engine is free before the first exp (the
            # v3 trace showed exp blocked until 22us by stalled
            # dma_start issues). q3's second half rides SWDGE (gpsimd),
            # gated on q1 via a 1-element dep chain so it cannot steal
            # early-stream bandwidth.
            nc.sync.dma_start(aux_sb[:], aux_d.ap())
            nc.sync.dma_start(bs_sb[:], bs_d.ap())
            nc.sync.dma_start(xk_sb[:, 0], xk_d.ap()[:, 0])
            nc.sync.dma_start(xq_sb[0][:], xq_d.ap()[0])
            nc.sync.dma_start(xv_sb[:], xv_d.ap())
            nc.sync.dma_start(xk_sb[:, 1], xk_d.ap()[:, 1])
            nc.sync.dma_start(xq_sb[1][:], xq_d.ap()[1])
            nc.sync.dma_start(xq_sb[2][:], xq_d.ap()[2])
            nc.sync.dma_start(xq_sb[3][:], xq_d.ap()[3])
            # The scalar (ACT) queue takes only 5 up-front dma_starts
            # (all retire early, in the scheduler's sim too, so exp
            # retirement isn't modeled late and the static tensor order
            # stays sane). q2b/vb dma_starts are dependency-gated to
            # slot between the first exps (see below); q3's second half
            # rides SWDGE (gpsimd), gated on q1's arrival. All gates are
            # 1-element poke chains: a byte of the DMA's destination is
            # rewritten after reading the gating tile, creating a WAW
            # dep the scheduler cannot reorder around.
            gpd = persist.tile([1, 4], f16)

            with tc.tile_pool(name="ps", bufs=1, space="PSUM") as ps_pool:
                # PE warmup trickle (HAM clock gate) covering the DMA fill
                wu_ps = ps_pool.tile([_P, 512], f32, name="wu", tag="av",
                                     bufs=2)
                nc.tensor.matmul(wu_ps[:_H, :], wu_sb[:, :_H], wu_sb[:],
                                 start=True, stop=True,
                                 skip_group_check=True)
                for _ in range(_NWU):
                    nc.tensor.matmul(wu_ps[:_H, :_P], wu_sb[:, :_H],
                                     wu_sb[:, :_P], start=True, stop=True,
                                     skip_group_check=True)
                # ACT exp table-set preload, hides under input DMA
                nc.scalar.activation(wu_act[:], wu_sb[:, :32], Exp)

                # Build duplicated weights [W|W] from the compact aux DMA
                for t in range(3):
                    for hf in range(2):
                        nc.vector.tensor_copy(
                            w2_sb[:, t, :, hf * 64:(hf + 1) * 64],
                            aux_sb[:, t * _EC:(t + 1) * _EC, :])

                def proj_dup(widx, rhs_of_e, c0, bias_t):
                    # one 512-token block; output duplicated to both
                    # partition halves via the [W|W] stationary operand
                    p = ps_pool.tile([_P, 512], f32, name="pj", tag="pj",
                                     bufs=2)
                    for e in range(_EC):
                        nc.tensor.matmul(
                            p[:], w2_sb[:, widx, e, :], rhs_of_e(e),
                            start=(e == 0), stop=(e == _EC - 1),
                        )
                    with tc.high_priority():
                        nc.vector.tensor_scalar_add(
                            kq2[:, c0:c0 + 512], p[:],
                            bs_sb[:, bias_t:bias_t + 1])

                def proj_v(bias_col):
                    # both 512-blocks of v concurrently (col-tiled pair)
                    p = ps_pool.tile([_P, 512], f32, name="pj", tag="pj",
                                     bufs=2)
                    for e in range(_EC):
                        nc.tensor.matmul(
                            p[0:64, :], w2_sb[:, 2, e, 0:64],
                            xv_sb[:, e, 0:512],
                            start=(e == 0), stop=(e == _EC - 1),
                            skip_group_check=True,
                        )
                        nc.tensor.matmul(
                            p[64:128, :], w2_sb[:, 2, e, 64:128],
                            xv_sb[:, e, 512:1024],
                            start=(e == 0), stop=(e == _EC - 1),
                            skip_group_check=True,
                        )
                    nc.vector.tensor_scalar_add(vT2[:], p[:], bias_col)

                def scores_exp(j, si):
                    # chunks 2si (rows 0:64) and 2si+1 (rows 64:128) as a
                    # row-tiled concurrent pair; high priority so the
                    # scheduler never parks these behind DMA-gated projs
                    # (the v4 trace showed sc02/03 statically ordered
                    # after q1/q2/v projections -> 10us ACT bubble)
                    qc = _SK + j * 512
                    with tc.high_priority():
                        sc = ps_pool.tile([_P, 1024], f32, name="sc",
                                          tag="sc", bufs=2)
                        nc.tensor.matmul(
                            sc[:, 0:512],
                            kq2[0:64, (2 * si) * _P:(2 * si + 1) * _P],
                            kq2[0:64, qc:qc + 512],
                            start=True, stop=True, skip_group_check=True,
                        )
                        nc.tensor.matmul(
                            sc[:, 512:1024],
                            kq2[64:128, (2 * si + 1) * _P:(2 * si + 2) * _P],
                            kq2[64:128, qc:qc + 512],
                            start=True, stop=True, skip_group_check=True,
                        )
                    # NOT high-priority: the ACTIVATE must stay behind the
                    # scalar-queue dma_start issues, or the scheduler
                    # defers those and the late input halves arrive at
                    # 40us+ (measured)
                    nc.scalar.activation(expT[j][:, si, :, :], sc[:], Exp)

                def av(j):
                    po = ps_pool.tile([_H + 1, 512], f32, name="po", tag="av",
                                      bufs=2, padded_shape=[_P, 512])
                    for c in range(_SKC):
                        nc.tensor.matmul(
                            po[:], v_sb[:, c, :],
                            expT[j][:, c // 2, c % 2, :],
                            start=(c == 0), stop=(c == _SKC - 1),
                        )
                    nc.vector.tensor_copy(oT_sb[:, j * 512:(j + 1) * 512],
                                          po[:])
                    eng = nc.scalar if j == 3 else nc.sync
                    eng.dma_start(out_d.ap()[:, j * 512:(j + 1) * 512],
                                  oT_sb[:, j * 512:(j + 1) * 512])

                bv = bs_sb[:, 2:3]

                def trickle():
                    # keep-warm: the exp-paced stretches idle the PE
                    # >3.4us and HAM re-throttles to 1.2GHz (measured) --
                    # a dummy N=128 matmul after each exp keeps 2.4GHz
                    nc.tensor.matmul(wu_ps[:_H, :_P], wu_sb[:, :_H],
                                     wu_sb[:, :_P], start=True, stop=True,
                                     skip_group_check=True)

                proj_dup(0, lambda e: xk_sb[:, 0, e, :], 0, 0)
                proj_dup(1, lambda e: xq_sb[0][:, e, :], _SK, 1)
                scores_exp(0, 0)
                scores_exp(0, 1)
                # v-proj fills the PE gap between the q0 chain and k1's
                # arrival (v is DMA'd right after q0)
                proj_v(bv)
                proj_dup(0, lambda e: xk_sb[:, 1, e, :], 512, 0)
                scores_exp(0, 2)
                scores_exp(0, 3)
                for j in (1, 2):
                    proj_dup(1, lambda e, j=j: xq_sb[j][:, e, :],
                             _SK + j * 512, 1)
                    for si in range(4):
                        scores_exp(j, si)
                        trickle()
                    if j == 2:
                        # PE-transposes of v slot here, right before the
                        # q3 chain, so the AVs behind them can weave
                        # into the exp-paced gaps
                        for c in range(_SKC):
                            pvt = ps_pool.tile([_P, _H], f16, name="pvt",
                                               tag="sc", bufs=2,
                                               padded_shape=[_P, 1024])
                            nc.tensor.transpose(
                                pvt[:],
                                vT2[(c // 4) * 64:(c // 4) * 64 + 64,
                                    (c % 4) * _P:(c % 4 + 1) * _P],
                                aux_sb[(c // 4) * 64:(c // 4) * 64 + 64,
                                       24, :])
                            nc.vector.tensor_copy(v_sb[:, c, :_H], pvt[:])

                proj_dup(1, lambda e: xq_sb[3][:, e, :], _SK + 3 * 512, 1)
                for si in range(4):
                    scores_exp(3, si)
                    trickle()

                av(0)
                av(1)
                av(2)
                av(3)

    nc.compile()
    return nc


def _prep_core(query, key, value, Wq, bq, Wk, bk, Wv, bv, core):
    b, half = core // 2, core % 2
    xkT = key[b].T[:, half * _SK:(half + 1) * _SK]         # [E, SK]
    xqT = query[b].T                                       # [E, S]
    xvT = value[b].T[:, half * _SK:(half + 1) * _SK]
    xk = xkT.reshape(_EC, _P, 2, 512).transpose(1, 2, 0, 3)  # [P, 2, EC, 512]
    xq = xqT.reshape(_EC, _P, _NQ, 512).transpose(2, 1, 0, 3)
    xv = xvT.reshape(_EC, _P, _SK).transpose(1, 0, 2)      # [P, EC, SK]
    w = np.stack(
        [Wk.reshape(_EC, _P, _H), Wq.reshape(_EC, _P, _H),
         Wv.reshape(_EC, _P, _H)], axis=0,
    ).transpose(2, 0, 1, 3)                                # [P, 3, EC, H]
    aux = np.concatenate(
        [w.reshape(_P, 24, 64),
         np.tile(np.eye(64, dtype=np.float16), (2, 1)).reshape(_P, 1, 64)],
        axis=1,
    )                                                      # [P, 25, 64]
    bs = np.tile(
        np.stack(
            [np.asarray(bk, dtype=np.float32).ravel(),
             np.asarray(bq, dtype=np.float32).ravel(),
             np.asarray(bv, dtype=np.float32).ravel()], axis=1,
        ),
        (2, 1),
    )                                                      # [P, 3]
    return {
        "aux": np.ascontiguousarray(aux).astype(np.float16),
        "bs": np.ascontiguousarray(bs, dtype=np.float32),
        "xk": np.ascontiguousarray(xk).astype(np.float16),
        "xq": np.ascontiguousarray(xq).astype(np.float16),
        "xv": np.ascontiguousarray(xv).astype(np.float16),
    }


def _get_built():
    global _built
    if _built is None:
        _built = _build()
    return _built


def kernel(query, key, value, Wq, bq, Wk, bk, Wv, bv, _trace=False):
    from concourse.bass_utils import run_bass_kernel_spmd

    query = np.asarray(query, dtype=np.float32)
    key = np.asarray(key, dtype=np.float32)
    value = np.asarray(value, dtype=np.float32)
    Wq = np.asarray(Wq, dtype=np.float32)
    Wk = np.asarray(Wk, dtype=np.float32)
    Wv = np.asarray(Wv, dtype=np.float32)

    nc = _get_built()
    in_maps = [
        _prep_core(query, key, value, Wq, bq, Wk, bk, Wv, bv, c) for c in range(8)
    ]
    res = run_bass_kernel_spmd(nc, in_maps, core_ids=list(range(8)), trace=_trace)
    out = np.empty((_B, _S, _H), dtype=np.float32)
    for b in range(_B):
        oA = res.results[2 * b]["out"]      # [H+1, S]
        oB = res.results[2 * b + 1]["out"]
        num = oA[: _H] + oB[: _H]
        den = oA[_H] + oB[_H]
        out[b] = (num / den).T
    if _trace:
        kernel.last_result = res
    return out
